# revision 1
# baseline (speedup 1.0000x reference)
"""Trainium2 Bass kernel for NeighborhoodNormalization.

Math: the reference builds a per-point homogeneous transform
T = [[ux,-uy,0,px],[uy,ux,0,py],[0,0,1,pz],[0,0,0,1]] (u = p/||p||),
inverts it, and applies it to 64 neighbors per point.  The inverse has a
closed form: with r2 = px^2+py^2, n = ||p||, a = n/r2, cx = px*a, cy = py*a:

    out.x =  cx*qx + cy*qy + tx      tx = -(cx*px + cy*py)
    out.y = -cy*qx + cx*qy + ty      ty =  (cy*px - cx*py)
    out.z =  qz - pz

So the kernel is pure elementwise math (memory-bound).  Sharding: pure data
parallel over the N=8192 point axis across 8 cores (1024 points/core).

Per-core layout: 16384 points = 128 partitions x 128 columns, where
partition p = b*8 + s holds points with local n = s*128 + t (t = column).
Neighbor rows (64*3 floats) stay contiguous in HBM per point, so DMAs are
[128 partitions x G*768B contiguous] blocks.  Per-point coefficients live as
[128,128] SBUF tiles; column t supplies the per-partition scalars for the
fused tensor_scalar / scalar_tensor_tensor / activation ops of column t.
"""

import sys

if "/opt/trn_rl_repo" not in sys.path:
    sys.path.insert(0, "/opt/trn_rl_repo")

import numpy as np

import concourse.bass as bass
import concourse.bacc as bacc
import concourse.mybir as mybir
from concourse.tile import TileContext
from concourse.bass_utils import run_bass_kernel_spmd

B = 16
N = 8192
K = 64
NCORES = 8
NLOC = N // NCORES  # 1024 points per core
P = 128             # SBUF partitions
S = NLOC // P       # 8 partition sub-blocks per batch entry
T = (B * NLOC) // P  # 128 point-columns per partition
G = 16              # columns per DMA group
NG = T // G

F32 = mybir.dt.float32
OP = mybir.AluOpType
AF = mybir.ActivationFunctionType

_CACHE = {}


def _build_nc():
    nc = bacc.Bacc(None, target_bir_lowering=False)

    pts = nc.declare_dram_parameter("points", [B, NLOC, 3], F32, isOutput=False)
    nb = nc.declare_dram_parameter("neighborhoods", [B, NLOC, K, 3], F32, isOutput=False)
    out = nc.declare_dram_parameter("out", [B, NLOC, K, 3], F32, isOutput=True)

    # partition = (b s), columns = t, free = 192 floats per point
    nbr = nb[:].rearrange("b (s t) k c -> (b s) t (k c)", s=S)
    outr = out[:].rearrange("b (s t) k c -> (b s) t (k c)", s=S)
    ptsr = pts[:].rearrange("b (s t) c -> (b s) (t c)", s=S)

    with TileContext(nc) as tc:
        with tc.tile_pool(name="const", bufs=1) as cpool, \
             tc.tile_pool(name="io_in", bufs=6) as inpool, \
             tc.tile_pool(name="io_out", bufs=6) as outpool, \
             tc.tile_pool(name="tmp", bufs=16) as tmppool:

            pts_sb = cpool.tile([P, T * 3], F32, tag="pts")
            nc.sync.dma_start(out=pts_sb[:], in_=ptsr)
            pv = pts_sb[:].rearrange("p (t c) -> p t c", c=3)
            px = pv[:, :, 0]
            py = pv[:, :, 1]
            pz = pv[:, :, 2]

            def ctile(tag):
                return cpool.tile([P, T], F32, tag=tag, name=tag)

            t1 = ctile("t1")
            t2 = ctile("t2")
            r2 = ctile("r2")
            n2 = ctile("n2")
            nn = ctile("nn")
            ir2 = ctile("ir2")
            aa = ctile("aa")
            cx = ctile("cx")
            cy = ctile("cy")
            ncy = ctile("ncy")
            tx = ctile("tx")
            ty = ctile("ty")
            npz = ctile("npz")

            nc.vector.tensor_mul(out=t1[:], in0=px, in1=px)
            nc.vector.tensor_mul(out=t2[:], in0=py, in1=py)
            nc.vector.tensor_add(out=r2[:], in0=t1[:], in1=t2[:])
            nc.vector.tensor_mul(out=t1[:], in0=pz, in1=pz)
            nc.vector.tensor_add(out=n2[:], in0=r2[:], in1=t1[:])
            nc.scalar.sqrt(out=nn[:], in_=n2[:])
            nc.vector.reciprocal(out=ir2[:], in_=r2[:])
            nc.vector.tensor_mul(out=aa[:], in0=nn[:], in1=ir2[:])
            nc.vector.tensor_mul(out=cx[:], in0=px, in1=aa[:])
            nc.vector.tensor_mul(out=cy[:], in0=py, in1=aa[:])
            nc.vector.tensor_scalar_mul(out=ncy[:], in0=cy[:], scalar1=-1.0)
            # tx = -(cx*px + cy*py)
            nc.vector.tensor_mul(out=t1[:], in0=cx[:], in1=px)
            nc.vector.tensor_mul(out=t2[:], in0=cy[:], in1=py)
            nc.vector.tensor_add(out=t1[:], in0=t1[:], in1=t2[:])
            nc.vector.tensor_scalar_mul(out=tx[:], in0=t1[:], scalar1=-1.0)
            # ty = cy*px - cx*py
            nc.vector.tensor_mul(out=t1[:], in0=cy[:], in1=px)
            nc.vector.tensor_mul(out=t2[:], in0=cx[:], in1=py)
            nc.vector.tensor_sub(out=ty[:], in0=t1[:], in1=t2[:])
            nc.vector.tensor_scalar_mul(out=npz[:], in0=pz, scalar1=-1.0)

            for g in range(NG):
                nb_t = inpool.tile([P, G, K, 3], F32, tag="nb", name=f"nb{g}")
                nc.sync.dma_start(
                    out=nb_t[:].rearrange("p g k c -> p g (k c)"),
                    in_=nbr[:, g * G:(g + 1) * G, :],
                )
                ot = outpool.tile([P, G, K, 3], F32, tag="ot", name=f"ot{g}")
                # out.z = qz - pz for the whole group in one wide op
                # (npz broadcast along the K axis via 0-stride AP)
                npz_b = npz[:, g * G:(g + 1) * G, None].broadcast_to([P, G, K])
                nc.vector.tensor_add(
                    out=ot[:, :, :, 2], in0=nb_t[:, :, :, 2], in1=npz_b,
                )
                for i in range(G):
                    t = g * G + i
                    qx = nb_t[:, i, :, 0]
                    qy = nb_t[:, i, :, 1]
                    ox = ot[:, i, :, 0]
                    oy = ot[:, i, :, 1]
                    cx_t = cx[:, t:t + 1]
                    cy_t = cy[:, t:t + 1]
                    ncy_t = ncy[:, t:t + 1]
                    tx_t = tx[:, t:t + 1]
                    ty_t = ty[:, t:t + 1]

                    # i2 = cy*qy + tx   (ACT: Identity(in*scale + bias))
                    tmp1 = tmppool.tile([P, K], F32, tag="tmp1", name=f"tmp1_{t}")
                    nc.scalar.activation(
                        out=tmp1[:], in_=qy, func=AF.Identity,
                        bias=tx_t, scale=cy_t,
                    )
                    # out.x = cx*qx + i2
                    nc.vector.scalar_tensor_tensor(
                        out=ox, in0=qx, scalar=cx_t, in1=tmp1[:],
                        op0=OP.mult, op1=OP.add,
                    )
                    # j2 = -cy*qx + ty   (GpSimd: otherwise idle)
                    tmp2 = tmppool.tile([P, K], F32, tag="tmp2", name=f"tmp2_{t}")
                    nc.gpsimd.tensor_scalar(
                        out=tmp2[:], in0=qx, scalar1=ncy_t, scalar2=ty_t,
                        op0=OP.mult, op1=OP.add,
                    )
                    # out.y = cx*qy + j2
                    nc.vector.scalar_tensor_tensor(
                        out=oy, in0=qy, scalar=cx_t, in1=tmp2[:],
                        op0=OP.mult, op1=OP.add,
                    )
                # out-DMA on the ACT HWDGE ring so it overlaps the SP-ring
                # input stream (HWDGE is FIFO per issuing engine).
                nc.scalar.dma_start(
                    out=outr[:, g * G:(g + 1) * G, :],
                    in_=ot[:].rearrange("p g k c -> p g (k c)"),
                )

    nc.compile()
    return nc


def _get_nc():
    if "nc" not in _CACHE:
        _CACHE["nc"] = _build_nc()
    return _CACHE["nc"]


def kernel(points, neighborhoods):
    pts = np.ascontiguousarray(np.asarray(points, dtype=np.float32))
    nb = np.ascontiguousarray(np.asarray(neighborhoods, dtype=np.float32))
    assert pts.shape == (B, N, 3), pts.shape
    assert nb.shape == (B, N, K, 3), nb.shape

    in_maps = []
    for c in range(NCORES):
        sl = slice(c * NLOC, (c + 1) * NLOC)
        in_maps.append({
            "points": np.ascontiguousarray(pts[:, sl]),
            "neighborhoods": np.ascontiguousarray(nb[:, sl]),
        })

    res = run_bass_kernel_spmd(_get_nc(), in_maps, list(range(NCORES))).results
    out = np.concatenate([res[c]["out"] for c in range(NCORES)], axis=1)
    return out



# revision 2
# speedup vs baseline: 1.2674x; 1.2674x over previous
"""Trainium2 Bass kernel for NeighborhoodNormalization (v2: bf16 wide-op).

Math: the reference builds a per-point homogeneous transform
T = [[ux,-uy,0,px],[uy,ux,0,py],[0,0,1,pz],[0,0,0,1]] (u = p/||p||),
inverts it, and applies it to 64 neighbors per point.  Closed form with
r2 = px^2+py^2, n = ||p||, a = n/r2, cx = px*a, cy = py*a, d = q - p:

    out.x =  cx*dx + cy*dy
    out.y = -cy*dx + cx*dy
    out.z =  dz

Pure elementwise math (memory-bound).  Sharding: data parallel over the
N=8192 point axis across 8 cores (1024 points/core).

Key layout choice: the host pre-splits neighborhoods into bf16 component
planes in k-major order [128 part=(b,s), K, T=128].  Every device op is
then a wide (FD=2048) step-1 bf16 tensor_tensor, which engages the DVE
2x_1p perf mode and amortizes the ~151-cycle per-instruction fixed cost
(the baseline's FD=64 per-column ops were ~70% fixed cost).  Per-point
coefficients are replicated across the KG k-slab once into REP tiles so
no operand needs a 0-stride broadcast AP (which would block 2x packing).
bf16 I/O also halves HBM traffic; rel-err budget (2e-2) is ~5x above the
~4e-3 this costs.
"""

import sys

if "/opt/trn_rl_repo" not in sys.path:
    sys.path.insert(0, "/opt/trn_rl_repo")

import numpy as np
import ml_dtypes

import concourse.bass as bass
import concourse.bacc as bacc
import concourse.mybir as mybir
from concourse.tile import TileContext
from concourse.bass_utils import run_bass_kernel_spmd

B = 16
N = 8192
K = 64
NCORES = 8
NLOC = N // NCORES   # 1024 points per core
P = 128              # SBUF partitions
S = 8                # sub-blocks per batch entry; partition = b*S + s
T = NLOC // S        # 128 points per partition row
KG = 16              # neighbors per DMA/compute group
NG = K // KG         # 4 groups

F32 = mybir.dt.float32
BF16 = mybir.dt.bfloat16
BF16_NP = ml_dtypes.bfloat16

_CACHE = {}


def _build_nc():
    nc = bacc.Bacc(None, target_bir_lowering=False)

    px = nc.declare_dram_parameter("px", [P, T], F32, isOutput=False)
    py = nc.declare_dram_parameter("py", [P, T], F32, isOutput=False)
    pz = nc.declare_dram_parameter("pz", [P, T], F32, isOutput=False)
    qx = nc.declare_dram_parameter("qx", [P, K, T], BF16, isOutput=False)
    qy = nc.declare_dram_parameter("qy", [P, K, T], BF16, isOutput=False)
    qz = nc.declare_dram_parameter("qz", [P, K, T], BF16, isOutput=False)
    ox = nc.declare_dram_parameter("ox", [P, K, T], BF16, isOutput=True)
    oy = nc.declare_dram_parameter("oy", [P, K, T], BF16, isOutput=True)
    oz = nc.declare_dram_parameter("oz", [P, K, T], BF16, isOutput=True)

    with TileContext(nc) as tc:
        with tc.tile_pool(name="const", bufs=1) as cpool, \
             tc.tile_pool(name="io_in", bufs=3) as inpool, \
             tc.tile_pool(name="io_out", bufs=3) as outpool, \
             tc.tile_pool(name="tmp", bufs=2) as tmppool:

            # --- per-point coefficients (f32, [P, T]) ---
            pxs = cpool.tile([P, T], F32, tag="pxs")
            pys = cpool.tile([P, T], F32, tag="pys")
            pzs = cpool.tile([P, T], F32, tag="pzs")
            nc.sync.dma_start(out=pxs[:], in_=px[:])
            nc.sync.dma_start(out=pys[:], in_=py[:])
            nc.sync.dma_start(out=pzs[:], in_=pz[:])

            def ctile(tag, dt=F32):
                return cpool.tile([P, T], dt, tag=tag, name=tag)

            t1 = ctile("t1")
            t2 = ctile("t2")
            r2 = ctile("r2")
            n2 = ctile("n2")
            nn = ctile("nn")
            ir2 = ctile("ir2")
            aa = ctile("aa")
            cxf = ctile("cxf")
            cyf = ctile("cyf")

            nc.vector.tensor_mul(out=t1[:], in0=pxs[:], in1=pxs[:])
            nc.vector.tensor_mul(out=t2[:], in0=pys[:], in1=pys[:])
            nc.vector.tensor_add(out=r2[:], in0=t1[:], in1=t2[:])
            nc.vector.tensor_mul(out=t1[:], in0=pzs[:], in1=pzs[:])
            nc.vector.tensor_add(out=n2[:], in0=r2[:], in1=t1[:])
            nc.scalar.sqrt(out=nn[:], in_=n2[:])
            nc.vector.reciprocal(out=ir2[:], in_=r2[:])
            nc.vector.tensor_mul(out=aa[:], in0=nn[:], in1=ir2[:])
            nc.vector.tensor_mul(out=cxf[:], in0=pxs[:], in1=aa[:])
            nc.vector.tensor_mul(out=cyf[:], in0=pys[:], in1=aa[:])

            # --- bf16 REP tiles [P, KG, T]: coefficient vectors replicated
            # across the k-slab so main-loop TT ops are all step-1 bf16 ---
            def rep(tag):
                return cpool.tile([P, KG, T], BF16, tag=tag, name=tag)

            pxr, pyr, pzr = rep("pxr"), rep("pyr"), rep("pzr")
            cxr, cyr = rep("cxr"), rep("cyr")
            nc.vector.tensor_copy(
                out=pxr[:], in_=pxs[:, None, :].broadcast_to([P, KG, T]))
            nc.vector.tensor_copy(
                out=pyr[:], in_=pys[:, None, :].broadcast_to([P, KG, T]))
            nc.scalar.copy(
                out=pzr[:], in_=pzs[:, None, :].broadcast_to([P, KG, T]))
            nc.scalar.copy(
                out=cxr[:], in_=cxf[:, None, :].broadcast_to([P, KG, T]))
            nc.scalar.copy(
                out=cyr[:], in_=cyf[:, None, :].broadcast_to([P, KG, T]))

            # --- main loop: 9 wide TT ops per k-slab group ---
            for g in range(NG):
                ks = slice(g * KG, (g + 1) * KG)
                qxt = inpool.tile([P, KG, T], BF16, tag="qx", name=f"qx{g}")
                qyt = inpool.tile([P, KG, T], BF16, tag="qy", name=f"qy{g}")
                qzt = inpool.tile([P, KG, T], BF16, tag="qz", name=f"qz{g}")
                nc.sync.dma_start(out=qxt[:], in_=qx[:][:, ks, :])
                nc.sync.dma_start(out=qyt[:], in_=qy[:][:, ks, :])
                nc.sync.dma_start(out=qzt[:], in_=qz[:][:, ks, :])

                oxt = outpool.tile([P, KG, T], BF16, tag="ox", name=f"ox{g}")
                oyt = outpool.tile([P, KG, T], BF16, tag="oy", name=f"oy{g}")
                ozt = outpool.tile([P, KG, T], BF16, tag="oz", name=f"oz{g}")

                dxt = tmppool.tile([P, KG, T], BF16, tag="dx", name=f"dx{g}")
                dyt = tmppool.tile([P, KG, T], BF16, tag="dy", name=f"dy{g}")
                m1 = tmppool.tile([P, KG, T], BF16, tag="m1", name=f"m1_{g}")
                m2 = tmppool.tile([P, KG, T], BF16, tag="m2", name=f"m2_{g}")
                m3 = tmppool.tile([P, KG, T], BF16, tag="m3", name=f"m3_{g}")
                m4 = tmppool.tile([P, KG, T], BF16, tag="m4", name=f"m4_{g}")

                # oz = qz - pz  (GpSimd; independent leaf)
                nc.gpsimd.tensor_sub(out=ozt[:], in0=qzt[:], in1=pzr[:])
                # d = q - p
                nc.vector.tensor_sub(out=dxt[:], in0=qxt[:], in1=pxr[:])
                nc.vector.tensor_sub(out=dyt[:], in0=qyt[:], in1=pyr[:])
                # ox = cx*dx + cy*dy
                nc.vector.tensor_mul(out=m1[:], in0=dxt[:], in1=cxr[:])
                nc.vector.tensor_mul(out=m2[:], in0=dyt[:], in1=cyr[:])
                nc.vector.tensor_add(out=oxt[:], in0=m1[:], in1=m2[:])
                # oy = cx*dy - cy*dx   (m3 on GpSimd)
                nc.gpsimd.tensor_mul(out=m3[:], in0=dxt[:], in1=cyr[:])
                nc.vector.tensor_mul(out=m4[:], in0=dyt[:], in1=cxr[:])
                nc.vector.tensor_sub(out=oyt[:], in0=m4[:], in1=m3[:])

                # out-DMAs on the ACT HWDGE ring (overlaps SP-ring input)
                nc.scalar.dma_start(out=ox[:][:, ks, :], in_=oxt[:])
                nc.scalar.dma_start(out=oy[:][:, ks, :], in_=oyt[:])
                nc.scalar.dma_start(out=oz[:][:, ks, :], in_=ozt[:])

    nc.compile()
    return nc


def _get_nc():
    if "nc" not in _CACHE:
        _CACHE["nc"] = _build_nc()
    return _CACHE["nc"]


def make_in_maps(points, neighborhoods):
    """Host-side sharding + layout: per core, f32 point component planes
    [P, T] and bf16 k-major neighbor component planes [P, K, T]."""
    pts = np.ascontiguousarray(np.asarray(points, dtype=np.float32))
    nb = np.asarray(neighborhoods, dtype=np.float32)
    assert pts.shape == (B, N, 3), pts.shape
    assert nb.shape == (B, N, K, 3), nb.shape

    nb_bf = nb.astype(BF16_NP)
    # [B, NCORES, S, T, K, 3]
    nb_r = nb_bf.reshape(B, NCORES, S, T, K, 3)
    pts_r = pts.reshape(B, NCORES, S, T, 3)

    in_maps = []
    for c in range(NCORES):
        m = {}
        for ci, name in enumerate(("px", "py", "pz")):
            m[name] = np.ascontiguousarray(
                pts_r[:, c, :, :, ci].reshape(P, T))
        for ci, name in enumerate(("qx", "qy", "qz")):
            # [B, S, T, K] -> [B*S, K, T]
            m[name] = np.ascontiguousarray(
                nb_r[:, c, :, :, :, ci].reshape(P, T, K).swapaxes(1, 2))
        in_maps.append(m)
    return in_maps


def assemble_out(results):
    """Merge per-core bf16 output planes back to [B, N, K, 3] f32."""
    out = np.empty((B, N, K, 3), dtype=np.float32)
    out_r = out.reshape(B, NCORES, S, T, K, 3)
    for c in range(NCORES):
        for ci, name in enumerate(("ox", "oy", "oz")):
            plane = results[c][name].reshape(P, K, T).swapaxes(1, 2)
            out_r[:, c, :, :, :, ci] = plane.reshape(B, S, T, K)
    return out


def kernel(points, neighborhoods):
    in_maps = make_in_maps(points, neighborhoods)
    res = run_bass_kernel_spmd(_get_nc(), in_maps, list(range(NCORES))).results
    return assemble_out(res)


# revision 6
# speedup vs baseline: 1.7621x; 1.3903x over previous
"""Trainium2 Bass kernel for NeighborhoodNormalization (v3: bf16 wide-op, DVE-only).

Math: the reference builds a per-point homogeneous transform
T = [[ux,-uy,0,px],[uy,ux,0,py],[0,0,1,pz],[0,0,0,1]] (u = p/||p||),
inverts it, and applies it to 64 neighbors per point.  Closed form with
r2 = px^2+py^2, n = ||p||, a = n/r2, cx = px*a, cy = py*a, d = q - p:

    out.x =  cx*dx + cy*dy
    out.y = -cy*dx + cx*dy
    out.z =  dz

Pure elementwise math (memory-bound).  Sharding: data parallel over the
N=8192 point axis across 8 cores (1024 points/core).

Layout: host pre-splits neighborhoods into bf16 planes in k-major order
[128 part=(b,s), K, ..., T=128] with x/y interleaved as T-runs, so every
device op is a wide (FD=2048/4096) step-1 bf16 tensor_tensor that engages
the DVE 2x_1p perf mode.  Per-point coefficient vectors are replicated
across the KG k-slab into REP tiles (built on ACT) so no compute operand
needs a 0-stride AP.  All main-loop compute runs on DVE only: GpSimd ops
arbitrate an exclusive shared SBUF port pair against DVE perf-mode ops
(loser fully blocks), so mixing engines is net-negative.  bf16 I/O halves
HBM traffic; rel-err cost (~4e-3) is 5x inside the 2e-2 budget.
"""

import sys

if "/opt/trn_rl_repo" not in sys.path:
    sys.path.insert(0, "/opt/trn_rl_repo")

import numpy as np
import ml_dtypes

import concourse.bass as bass
import concourse.bacc as bacc
import concourse.mybir as mybir
from concourse.tile import TileContext
from concourse.bass_utils import run_bass_kernel_spmd

B = 16
N = 8192
K = 64
NCORES = 8
NLOC = N // NCORES   # 1024 points per core
P = 128              # SBUF partitions
S = 8                # sub-blocks per batch entry; partition = b*S + s
T = NLOC // S        # 128 points per partition row
KG = 16              # neighbors per DMA/compute group
NG = K // KG         # 4 groups

# Use REP tiles (True) vs 0-stride broadcast APs (False) for coefficients.
USE_REP = True

F32 = mybir.dt.float32
BF16 = mybir.dt.bfloat16
BF16_NP = ml_dtypes.bfloat16

_CACHE = {}


def _build_nc():
    nc = bacc.Bacc(None, target_bir_lowering=False)

    px = nc.declare_dram_parameter("px", [P, T], F32, isOutput=False)
    py = nc.declare_dram_parameter("py", [P, T], F32, isOutput=False)
    pz = nc.declare_dram_parameter("pz", [P, T], F32, isOutput=False)
    qxy = nc.declare_dram_parameter("qxy", [P, K, 2, T], BF16, isOutput=False)
    qz = nc.declare_dram_parameter("qz", [P, K, T], BF16, isOutput=False)
    oxy = nc.declare_dram_parameter("oxy", [P, K, 2, T], BF16, isOutput=True)
    oz = nc.declare_dram_parameter("oz", [P, K, T], BF16, isOutput=True)

    with TileContext(nc) as tc:
        with tc.tile_pool(name="const", bufs=1) as cpool, \
             tc.tile_pool(name="io_in", bufs=3) as inpool, \
             tc.tile_pool(name="io_out", bufs=3) as outpool, \
             tc.tile_pool(name="tmp", bufs=2) as tmppool:

            # --- per-point coefficients (f32, [P, T]) ---
            pxs = cpool.tile([P, T], F32, tag="pxs")
            pys = cpool.tile([P, T], F32, tag="pys")
            pzs = cpool.tile([P, T], F32, tag="pzs")
            nc.sync.dma_start(out=pxs[:], in_=px[:])
            nc.sync.dma_start(out=pys[:], in_=py[:])
            nc.sync.dma_start(out=pzs[:], in_=pz[:])

            def ctile(tag, dt=F32):
                return cpool.tile([P, T], dt, tag=tag, name=tag)

            t1 = ctile("t1")
            t2 = ctile("t2")
            t3 = ctile("t3")
            r2 = ctile("r2")
            n2 = ctile("n2")
            nn = ctile("nn")
            ir2 = ctile("ir2")
            aa = ctile("aa")
            cxf = ctile("cxf")
            cyf = ctile("cyf")

            # squares + sqrt on ACT, rest on DVE (f32, small)
            nc.scalar.square(out=t1[:], in_=pxs[:])
            nc.scalar.square(out=t2[:], in_=pys[:])
            nc.scalar.square(out=t3[:], in_=pzs[:])
            nc.vector.tensor_add(out=r2[:], in0=t1[:], in1=t2[:])
            nc.vector.tensor_add(out=n2[:], in0=r2[:], in1=t3[:])
            nc.scalar.sqrt(out=nn[:], in_=n2[:])
            nc.vector.reciprocal(out=ir2[:], in_=r2[:])
            nc.vector.tensor_mul(out=aa[:], in0=nn[:], in1=ir2[:])
            nc.vector.tensor_mul(out=cxf[:], in0=pxs[:], in1=aa[:])
            nc.vector.tensor_mul(out=cyf[:], in0=pys[:], in1=aa[:])

            # --- bf16 REP tiles: coefficient vectors replicated across the
            # k-slab (and x/y-interleaved as T-runs), built on ACT so DVE
            # stays free.  Order matters: p-REPs first (only need pts DMA),
            # c-REPs after the coefficient chain. ---
            pxyr = cpool.tile([P, KG, 2, T], BF16, tag="pxyr")
            cxyr = cpool.tile([P, KG, 2, T], BF16, tag="cxyr")
            cyxr = cpool.tile([P, KG, 2, T], BF16, tag="cyxr")
            pzr = cpool.tile([P, KG, T], BF16, tag="pzr")

            def bcast(src):
                return src[:, None, :].broadcast_to([P, KG, T])

            nc.scalar.copy(out=pxyr[:, :, 0, :], in_=bcast(pxs))
            nc.scalar.copy(out=pxyr[:, :, 1, :], in_=bcast(pys))
            nc.scalar.copy(out=pzr[:], in_=bcast(pzs))
            nc.scalar.copy(out=cxyr[:, :, 0, :], in_=bcast(cxf))
            nc.scalar.copy(out=cxyr[:, :, 1, :], in_=bcast(cyf))
            nc.scalar.copy(out=cyxr[:, :, 0, :], in_=bcast(cyf))
            nc.scalar.copy(out=cyxr[:, :, 1, :], in_=bcast(cxf))
            pxy_in = pxyr[:]
            cxy_in = cxyr[:]
            cyx_in = cyxr[:]
            pz_in = pzr[:]

            # --- main loop: 6 wide DVE ops per k-slab group ---
            for g in range(NG):
                ks = slice(g * KG, (g + 1) * KG)
                qxyt = inpool.tile([P, KG, 2, T], BF16, tag="qxy",
                                   name=f"qxy{g}")
                qzt = inpool.tile([P, KG, T], BF16, tag="qz", name=f"qz{g}")
                nc.sync.dma_start(out=qxyt[:], in_=qxy[:][:, ks, :, :])
                nc.sync.dma_start(out=qzt[:], in_=qz[:][:, ks, :])

                oxyt = outpool.tile([P, KG, 2, T], BF16, tag="oxy",
                                    name=f"oxy{g}")
                ozt = outpool.tile([P, KG, T], BF16, tag="oz", name=f"oz{g}")

                dxy = tmppool.tile([P, KG, 2, T], BF16, tag="dxy",
                                   name=f"dxy{g}")
                u = tmppool.tile([P, KG, 2, T], BF16, tag="u", name=f"u{g}")
                v = tmppool.tile([P, KG, 2, T], BF16, tag="v", name=f"v{g}")

                # d = q - p   (covers both x and y runs)
                nc.vector.tensor_sub(out=dxy[:], in0=qxyt[:], in1=pxy_in)
                # u = (cx*dx | cy*dy);  ox = u0 + u1
                nc.vector.tensor_mul(out=u[:], in0=dxy[:], in1=cxy_in)
                nc.vector.tensor_add(
                    out=oxyt[:, :, 0, :], in0=u[:, :, 0, :], in1=u[:, :, 1, :])
                # v = (cy*dx | cx*dy);  oy = v1 - v0
                nc.vector.tensor_mul(out=v[:], in0=dxy[:], in1=cyx_in)
                nc.vector.tensor_sub(
                    out=oxyt[:, :, 1, :], in0=v[:, :, 1, :], in1=v[:, :, 0, :])
                # oz = qz - pz
                nc.vector.tensor_sub(out=ozt[:], in0=qzt[:], in1=pz_in)

                # out-DMAs on the ACT HWDGE ring (overlaps SP-ring input)
                nc.scalar.dma_start(out=oxy[:][:, ks, :, :], in_=oxyt[:])
                nc.scalar.dma_start(out=oz[:][:, ks, :], in_=ozt[:])

    nc.compile()
    return nc


def _get_nc():
    if "nc" not in _CACHE:
        _CACHE["nc"] = _build_nc()
    return _CACHE["nc"]


def make_in_maps(points, neighborhoods):
    """Host-side sharding + layout: per core, f32 point component planes
    [P, T], bf16 k-major xy-paired plane [P, K, 2, T] and z plane
    [P, K, T]."""
    pts = np.ascontiguousarray(np.asarray(points, dtype=np.float32))
    nb = np.asarray(neighborhoods, dtype=np.float32)
    assert pts.shape == (B, N, 3), pts.shape
    assert nb.shape == (B, N, K, 3), nb.shape

    nb_bf = nb.astype(BF16_NP)
    # [B, NCORES, S, T, K, 3]
    nb_r = nb_bf.reshape(B, NCORES, S, T, K, 3)
    pts_r = pts.reshape(B, NCORES, S, T, 3)

    in_maps = []
    for c in range(NCORES):
        m = {}
        for ci, name in enumerate(("px", "py", "pz")):
            m[name] = np.ascontiguousarray(
                pts_r[:, c, :, :, ci].reshape(P, T))
        # [B, S, T, K, 2] -> [P, K, 2, T]
        m["qxy"] = np.ascontiguousarray(
            nb_r[:, c, :, :, :, 0:2].reshape(P, T, K, 2).transpose(0, 2, 3, 1))
        m["qz"] = np.ascontiguousarray(
            nb_r[:, c, :, :, :, 2].reshape(P, T, K).swapaxes(1, 2))
        in_maps.append(m)
    return in_maps


def assemble_out(results):
    """Merge per-core bf16 output planes back to [B, N, K, 3] f32."""
    out = np.empty((B, N, K, 3), dtype=np.float32)
    out_r = out.reshape(B, NCORES, S, T, K, 3)
    for c in range(NCORES):
        # [P, K, 2, T] -> [B, S, T, K, 2]
        oxy = results[c]["oxy"].reshape(P, K, 2, T).transpose(0, 3, 1, 2)
        out_r[:, c, :, :, :, 0:2] = oxy.reshape(B, S, T, K, 2)
        ozp = results[c]["oz"].reshape(P, K, T).swapaxes(1, 2)
        out_r[:, c, :, :, :, 2] = ozp.reshape(B, S, T, K)
    return out


def kernel(points, neighborhoods):
    in_maps = make_in_maps(points, neighborhoods)
    res = run_bass_kernel_spmd(_get_nc(), in_maps, list(range(NCORES))).results
    return assemble_out(res)


# revision 7
# speedup vs baseline: 1.7881x; 1.0148x over previous
"""Trainium2 Bass kernel for NeighborhoodNormalization (v4).

Math: the reference builds a per-point homogeneous transform
T = [[ux,-uy,0,px],[uy,ux,0,py],[0,0,1,pz],[0,0,0,1]] (u = p/||p||),
inverts it, and applies it to 64 neighbors per point.  Closed form with
r2 = px^2+py^2, n = ||p||, a = n/r2, cx = px*a, cy = py*a, d = q - p:

    out.x =  cx*dx + cy*dy
    out.y = -cy*dx + cx*dy
    out.z =  dz

Pure elementwise math (memory-bound).  Sharding: data parallel over the
N=8192 point axis across 8 cores (1024 points/core).

Engine plan (v4):
- DVE: the x/y rotation only, as wide (FD=2048/4096) step-1 bf16
  tensor_tensor ops in 2x_1p perf mode.  Host pre-splits neighborhoods
  into bf16 planes in k-major order [128 part=(b,s), K, 2, T=128] with
  x/y interleaved as T-runs so every op is step-1.  GpSimd stays idle:
  its ops arbitrate an exclusive shared SBUF port pair against DVE
  perf-mode ops (the loser fully blocks), so mixing engines loses.
- PE+ACT: the z plane (oz = qz - pz) via PSUM accumulation of
  I @ qz + (-I) @ pz_rep, ACT copies PSUM->SBUF (with bf16 cast).  Both
  engines have their own SBUF ports, so this is free parallelism.
- ACT also builds the REP tiles (coefficient vectors replicated across
  the k-slab) ordered so the p-REPs (needed by the first DVE op) come
  before the coefficient chain.
bf16 I/O halves HBM traffic; rel-err cost (~4e-3) is 5x inside 2e-2.
"""

import sys

if "/opt/trn_rl_repo" not in sys.path:
    sys.path.insert(0, "/opt/trn_rl_repo")

import numpy as np
import ml_dtypes

import concourse.bass as bass
import concourse.bacc as bacc
import concourse.mybir as mybir
from concourse.tile import TileContext
from concourse.bass_utils import run_bass_kernel_spmd

B = 16
N = 8192
K = 64
NCORES = 8
NLOC = N // NCORES   # 1024 points per core
P = 128              # SBUF partitions
S = 8                # sub-blocks per batch entry; partition = b*S + s
T = NLOC // S        # 128 points per partition row
KG = 16              # max neighbors per DMA/compute group
GROUPS = [16, 16, 16, 8, 8]   # tapered tail for faster drain
assert sum(GROUPS) == K
ZC = 512             # z-plane PSUM chunk (one 2KB bank of f32)

F32 = mybir.dt.float32
BF16 = mybir.dt.bfloat16
BF16_NP = ml_dtypes.bfloat16

_CACHE = {}


def _build_nc():
    nc = bacc.Bacc(None, target_bir_lowering=False)

    px = nc.declare_dram_parameter("px", [P, T], F32, isOutput=False)
    py = nc.declare_dram_parameter("py", [P, T], F32, isOutput=False)
    pz = nc.declare_dram_parameter("pz", [P, T], F32, isOutput=False)
    idp = nc.declare_dram_parameter("idp", [P, P], BF16, isOutput=False)
    idm = nc.declare_dram_parameter("idm", [P, P], BF16, isOutput=False)
    qxy = nc.declare_dram_parameter("qxy", [P, K, 2, T], BF16, isOutput=False)
    qz = nc.declare_dram_parameter("qz", [P, K, T], BF16, isOutput=False)
    oxy = nc.declare_dram_parameter("oxy", [P, K, 2, T], BF16, isOutput=True)
    oz = nc.declare_dram_parameter("oz", [P, K, T], BF16, isOutput=True)

    with TileContext(nc) as tc:
        with tc.tile_pool(name="const", bufs=1) as cpool, \
             tc.tile_pool(name="io_in", bufs=3) as inpool, \
             tc.tile_pool(name="io_out", bufs=3) as outpool, \
             tc.tile_pool(name="tmp", bufs=3) as tmppool, \
             tc.tile_pool(name="zps", bufs=4, space="PSUM") as zpool:

            # --- point component vectors + identity stationaries ---
            pxs = cpool.tile([P, T], F32, tag="pxs")
            pys = cpool.tile([P, T], F32, tag="pys")
            pzs = cpool.tile([P, T], F32, tag="pzs")
            idpt = cpool.tile([P, P], BF16, tag="idp")
            idmt = cpool.tile([P, P], BF16, tag="idm")
            nc.sync.dma_start(out=pxs[:], in_=px[:])
            nc.sync.dma_start(out=pys[:], in_=py[:])
            nc.sync.dma_start(out=pzs[:], in_=pz[:])
            nc.sync.dma_start(out=idpt[:], in_=idp[:])
            nc.sync.dma_start(out=idmt[:], in_=idm[:])

            # --- REP tiles: coefficient vectors replicated across the
            # k-slab as x/y-interleaved T-runs.  Built on ACT; p-REPs
            # first (they gate the first DVE op of every group). ---
            pxyr = cpool.tile([P, KG, 2, T], BF16, tag="pxyr")
            cxyr = cpool.tile([P, KG, 2, T], BF16, tag="cxyr")
            cyxr = cpool.tile([P, KG, 2, T], BF16, tag="cyxr")
            pzr = cpool.tile([P, KG, T], BF16, tag="pzr")

            def bcast(src):
                return src[:, None, :].broadcast_to([P, KG, T])

            nc.scalar.copy(out=pxyr[:, :, 0, :], in_=bcast(pxs))
            nc.scalar.copy(out=pxyr[:, :, 1, :], in_=bcast(pys))
            nc.scalar.copy(out=pzr[:], in_=bcast(pzs))

            # --- coefficient chain (small f32; ACT squares/sqrt, DVE rest) ---
            def ctile(tag, dt=F32):
                return cpool.tile([P, T], dt, tag=tag, name=tag)

            t1 = ctile("t1")
            t2 = ctile("t2")
            t3 = ctile("t3")
            r2 = ctile("r2")
            n2 = ctile("n2")
            nn = ctile("nn")
            ir2 = ctile("ir2")
            aa = ctile("aa")
            cxf = ctile("cxf")
            cyf = ctile("cyf")

            nc.scalar.square(out=t1[:], in_=pxs[:])
            nc.scalar.square(out=t2[:], in_=pys[:])
            nc.scalar.square(out=t3[:], in_=pzs[:])
            nc.vector.tensor_add(out=r2[:], in0=t1[:], in1=t2[:])
            nc.vector.tensor_add(out=n2[:], in0=r2[:], in1=t3[:])
            nc.scalar.sqrt(out=nn[:], in_=n2[:])
            nc.vector.reciprocal(out=ir2[:], in_=r2[:])
            nc.vector.tensor_mul(out=aa[:], in0=nn[:], in1=ir2[:])
            nc.vector.tensor_mul(out=cxf[:], in0=pxs[:], in1=aa[:])
            nc.vector.tensor_mul(out=cyf[:], in0=pys[:], in1=aa[:])

            nc.scalar.copy(out=cxyr[:, :, 0, :], in_=bcast(cxf))
            nc.scalar.copy(out=cxyr[:, :, 1, :], in_=bcast(cyf))
            nc.scalar.copy(out=cyxr[:, :, 0, :], in_=bcast(cyf))
            nc.scalar.copy(out=cyxr[:, :, 1, :], in_=bcast(cxf))

            # --- main loop ---
            k0 = 0
            for g, kg in enumerate(GROUPS):
                ks = slice(k0, k0 + kg)
                k0 += kg
                qxyt = inpool.tile([P, KG, 2, T], BF16, tag="qxy",
                                   name=f"qxy{g}")[:, :kg]
                qzt = inpool.tile([P, KG, T], BF16, tag="qz",
                                  name=f"qz{g}")[:, :kg]
                nc.sync.dma_start(out=qxyt, in_=qxy[:][:, ks, :, :])
                nc.sync.dma_start(out=qzt, in_=qz[:][:, ks, :])

                oxyt = outpool.tile([P, KG, 2, T], BF16, tag="oxy",
                                    name=f"oxy{g}")[:, :kg]
                ozt = outpool.tile([P, KG, T], BF16, tag="oz",
                                   name=f"oz{g}")[:, :kg]

                dxy = tmppool.tile([P, KG, 2, T], BF16, tag="dxy",
                                   name=f"dxy{g}")[:, :kg]
                u = tmppool.tile([P, KG, 2, T], BF16, tag="u",
                                 name=f"u{g}")[:, :kg]
                v = tmppool.tile([P, KG, 2, T], BF16, tag="v",
                                 name=f"v{g}")[:, :kg]

                # --- x/y rotation on DVE (all step-1 bf16, 2x mode) ---
                nc.vector.tensor_sub(out=dxy, in0=qxyt, in1=pxyr[:, :kg])
                nc.vector.tensor_mul(out=u, in0=dxy, in1=cxyr[:, :kg])
                nc.vector.tensor_add(
                    out=oxyt[:, :, 0, :], in0=u[:, :, 0, :], in1=u[:, :, 1, :])
                nc.vector.tensor_mul(out=v, in0=dxy, in1=cyxr[:, :kg])
                nc.vector.tensor_sub(
                    out=oxyt[:, :, 1, :], in0=v[:, :, 1, :], in1=v[:, :, 0, :])

                # --- z plane on PE (PSUM: I@qz + (-I)@pz_rep) + ACT copy ---
                zflat = qzt.rearrange("p k t -> p (k t)")
                ozflat = ozt.rearrange("p k t -> p (k t)")
                pzflat = pzr[:].rearrange("p k t -> p (k t)")
                nchunk = (kg * T) // ZC
                for ci in range(nchunk):
                    cs = slice(ci * ZC, (ci + 1) * ZC)
                    ps = zpool.tile([P, ZC], F32, tag="zp", name=f"zp{g}_{ci}")
                    nc.tensor.matmul(ps[:], idpt[:], zflat[:, cs],
                                     start=True, stop=False)
                    nc.tensor.matmul(ps[:], idmt[:], pzflat[:, cs],
                                     start=False, stop=True)
                    nc.scalar.copy(out=ozflat[:, cs], in_=ps[:])

                # out-DMAs on the ACT HWDGE ring (overlaps SP-ring input)
                nc.scalar.dma_start(out=oz[:][:, ks, :], in_=ozt)
                nc.scalar.dma_start(out=oxy[:][:, ks, :, :], in_=oxyt)

    nc.compile()
    return nc


def _get_nc():
    if "nc" not in _CACHE:
        _CACHE["nc"] = _build_nc()
    return _CACHE["nc"]


_EYE = None


def make_in_maps(points, neighborhoods):
    """Host-side sharding + layout: per core, f32 point component planes
    [P, T], bf16 k-major xy-paired plane [P, K, 2, T], z plane [P, K, T],
    and +-identity matmul stationaries."""
    global _EYE
    pts = np.ascontiguousarray(np.asarray(points, dtype=np.float32))
    nb = np.asarray(neighborhoods, dtype=np.float32)
    assert pts.shape == (B, N, 3), pts.shape
    assert nb.shape == (B, N, K, 3), nb.shape

    if _EYE is None:
        _EYE = np.eye(P, dtype=BF16_NP)
    nb_bf = nb.astype(BF16_NP)
    # [B, NCORES, S, T, K, 3]
    nb_r = nb_bf.reshape(B, NCORES, S, T, K, 3)
    pts_r = pts.reshape(B, NCORES, S, T, 3)

    in_maps = []
    for c in range(NCORES):
        m = {"idp": _EYE, "idm": -_EYE}
        for ci, name in enumerate(("px", "py", "pz")):
            m[name] = np.ascontiguousarray(
                pts_r[:, c, :, :, ci].reshape(P, T))
        # [B, S, T, K, 2] -> [P, K, 2, T]
        m["qxy"] = np.ascontiguousarray(
            nb_r[:, c, :, :, :, 0:2].reshape(P, T, K, 2).transpose(0, 2, 3, 1))
        m["qz"] = np.ascontiguousarray(
            nb_r[:, c, :, :, :, 2].reshape(P, T, K).swapaxes(1, 2))
        in_maps.append(m)
    return in_maps


def assemble_out(results):
    """Merge per-core bf16 output planes back to [B, N, K, 3] f32."""
    out = np.empty((B, N, K, 3), dtype=np.float32)
    out_r = out.reshape(B, NCORES, S, T, K, 3)
    for c in range(NCORES):
        # [P, K, 2, T] -> [B, S, T, K, 2]
        oxy = results[c]["oxy"].reshape(P, K, 2, T).transpose(0, 3, 1, 2)
        out_r[:, c, :, :, :, 0:2] = oxy.reshape(B, S, T, K, 2)
        ozp = results[c]["oz"].reshape(P, K, T).swapaxes(1, 2)
        out_r[:, c, :, :, :, 2] = ozp.reshape(B, S, T, K)
    return out


def kernel(points, neighborhoods):
    in_maps = make_in_maps(points, neighborhoods)
    res = run_bass_kernel_spmd(_get_nc(), in_maps, list(range(NCORES))).results
    return assemble_out(res)


# revision 10
# speedup vs baseline: 1.8766x; 1.0495x over previous
"""Trainium2 Bass kernel for NeighborhoodNormalization (v5).

Math: the reference builds a per-point homogeneous transform
T = [[ux,-uy,0,px],[uy,ux,0,py],[0,0,1,pz],[0,0,0,1]] (u = p/||p||),
inverts it, and applies it to 64 neighbors per point.  With
r2 = px^2+py^2, n = ||p||, cx = px*n/r2, cy = py*n/r2 the inverse's
translation collapses: -(cx*px+cy*py) = -n and cy*px-cx*py = 0, so

    out.x =  cx*qx + cy*qy - n
    out.y = -cy*qx + cx*qy
    out.z =  qz - pz

Sharding: data parallel over the N=8192 point axis across 8 cores.

Engine plan (v5):
- Host pre-splits neighborhoods into bf16 planes in k-major order
  [128 part=(b,s), K, {2,}, T=128] (x/y interleaved as T-runs) so every
  DVE op is wide, step-1 bf16 -> 2x_1p perf mode.
- DVE: the two products u=(cx*qx|cy*qy), v=(-cy*qx|cx*qy) per group,
  plus the small f32 coefficient chain and the bf16 REP tiles
  (coefficient vectors replicated across the k-slab via 2x_2p
  broadcast-copies).  GpSimd stays idle (its ops arbitrate an exclusive
  shared SBUF port pair against DVE perf-mode ops; the loser blocks).
- PE: all combines as PSUM accumulations with a single identity
  stationary (x: I@u0+I@u1+I@txr, y: I@v0+I@v1, z: I@qz+I@npzr) in
  512-col bank chunks (~120ns/matmul incl LDWEIGHTS).
- ACT: sqrt (table preloaded via a dummy), PSUM->SBUF evictions with
  bf16 cast, out-DMA triggers on its HWDGE ring.
bf16 I/O halves HBM traffic; rel-err cost (~3.3e-3) is 6x inside 2e-2.
"""

import sys

if "/opt/trn_rl_repo" not in sys.path:
    sys.path.insert(0, "/opt/trn_rl_repo")

import numpy as np
import ml_dtypes

import concourse.bass as bass
import concourse.bacc as bacc
import concourse.mybir as mybir
from concourse.tile import TileContext
from concourse.bass_utils import run_bass_kernel_spmd

B = 16
N = 8192
K = 64
NCORES = 8
NLOC = N // NCORES   # 1024 points per core
P = 128              # SBUF partitions
S = 8                # sub-blocks per batch entry; partition = b*S + s
T = NLOC // S        # 128 points per partition row
KG = 16              # max neighbors per DMA/compute group
GROUPS = [16, 16, 16, 8, 8]   # tapered tail for faster drain
assert sum(GROUPS) == K
BANK = 512           # PSUM bank, f32 elems; matmul out chunk
PST = 1024           # psum tile width (2 banks)

F32 = mybir.dt.float32
BF16 = mybir.dt.bfloat16
BF16_NP = ml_dtypes.bfloat16

_CACHE = {}


def _build_nc():
    nc = bacc.Bacc(None, target_bir_lowering=False)

    px = nc.declare_dram_parameter("px", [P, T], F32, isOutput=False)
    py = nc.declare_dram_parameter("py", [P, T], F32, isOutput=False)
    pz = nc.declare_dram_parameter("pz", [P, T], F32, isOutput=False)
    idp = nc.declare_dram_parameter("idp", [P, P], BF16, isOutput=False)
    qxy = nc.declare_dram_parameter("qxy", [P, K, 2, T], BF16, isOutput=False)
    qz = nc.declare_dram_parameter("qz", [P, K, T], BF16, isOutput=False)
    oxy = nc.declare_dram_parameter("oxy", [P, K, 2, T], BF16, isOutput=True)
    oz = nc.declare_dram_parameter("oz", [P, K, T], BF16, isOutput=True)

    with TileContext(nc) as tc:
        with tc.tile_pool(name="const", bufs=1) as cpool, \
             tc.tile_pool(name="io_in", bufs=3) as inpool, \
             tc.tile_pool(name="io_out", bufs=3) as outpool, \
             tc.tile_pool(name="tmp", bufs=3) as tmppool, \
             tc.tile_pool(name="zps", bufs=4, space="PSUM") as zpool:

            # dummy sqrt first so ACT's sqrt table is hot before n2 lands
            dumt = cpool.tile([P, 1], F32, tag="dum")
            dumo = cpool.tile([P, 1], F32, tag="dumo")
            nc.gpsimd.memset(dumt[:], 0.0)
            nc.scalar.sqrt(out=dumo[:], in_=dumt[:])

            pxs = cpool.tile([P, T], F32, tag="pxs")
            pys = cpool.tile([P, T], F32, tag="pys")
            pzs = cpool.tile([P, T], F32, tag="pzs")
            idpt = cpool.tile([P, P], BF16, tag="idp")
            nc.sync.dma_start(out=pxs[:], in_=px[:])
            nc.sync.dma_start(out=pys[:], in_=py[:])
            nc.sync.dma_start(out=pzs[:], in_=pz[:])
            nc.sync.dma_start(out=idpt[:], in_=idp[:])

            # --- coefficient chain (f32 [P, T], DVE + one ACT sqrt) ---
            def ctile(tag, dt=F32):
                return cpool.tile([P, T], dt, tag=tag, name=tag)

            t1 = ctile("t1")
            t2 = ctile("t2")
            t3 = ctile("t3")
            r2 = ctile("r2")
            n2 = ctile("n2")
            nn = ctile("nn")
            ir2 = ctile("ir2")
            aa = ctile("aa")
            cxf = ctile("cxf")
            cyf = ctile("cyf")

            nc.vector.tensor_mul(out=t1[:], in0=pxs[:], in1=pxs[:])
            nc.vector.tensor_mul(out=t2[:], in0=pys[:], in1=pys[:])
            nc.vector.tensor_add(out=r2[:], in0=t1[:], in1=t2[:])
            nc.vector.tensor_mul(out=t3[:], in0=pzs[:], in1=pzs[:])
            nc.vector.tensor_add(out=n2[:], in0=r2[:], in1=t3[:])
            nc.scalar.sqrt(out=nn[:], in_=n2[:])
            nc.vector.reciprocal(out=ir2[:], in_=r2[:])
            nc.vector.tensor_mul(out=aa[:], in0=nn[:], in1=ir2[:])
            nc.vector.tensor_mul(out=cxf[:], in0=pxs[:], in1=aa[:])
            nc.vector.tensor_mul(out=cyf[:], in0=pys[:], in1=aa[:])

            # --- REP tiles (bf16, coefficient vectors replicated across
            # the k-slab; built on DVE via 2x_2p broadcast-copies) ---
            cxyr = cpool.tile([P, KG, 2, T], BF16, tag="cxyr")
            cyxr = cpool.tile([P, KG, 2, T], BF16, tag="cyxr")  # (-cy | cx)
            txr = cpool.tile([P, KG, T], BF16, tag="txr")       # -n
            npzr = cpool.tile([P, KG, T], BF16, tag="npzr")     # -pz

            def bcast(src):
                return src[:, None, :].broadcast_to([P, KG, T])

            groups = []
            k0 = 0
            for g, kg in enumerate(GROUPS):
                ks = slice(k0, k0 + kg)
                k0 += kg
                qxyt = inpool.tile([P, KG, 2, T], BF16, tag="qxy",
                                   name=f"qxy{g}")[:, :kg]
                qzt = inpool.tile([P, KG, T], BF16, tag="qz",
                                  name=f"qz{g}")[:, :kg]
                nc.sync.dma_start(out=qxyt, in_=qxy[:][:, ks, :, :])
                nc.sync.dma_start(out=qzt, in_=qz[:][:, ks, :])
                groups.append((g, kg, ks, qxyt, qzt))

            # DVE stream: REP builds interleaved with the first group's
            # products so nothing downstream waits longer than it must.
            uvs = []
            nc.vector.tensor_copy(out=cxyr[:, :, 0, :], in_=bcast(cxf))
            nc.vector.tensor_copy(out=cxyr[:, :, 1, :], in_=bcast(cyf))

            def emit_u(g, kg, qxyt):
                u = tmppool.tile([P, KG, 2, T], BF16, tag="u",
                                 name=f"u{g}")[:, :kg]
                nc.vector.tensor_mul(out=u, in0=qxyt, in1=cxyr[:, :kg])
                return u

            def emit_v(g, kg, qxyt):
                v = tmppool.tile([P, KG, 2, T], BF16, tag="v",
                                 name=f"v{g}")[:, :kg]
                nc.vector.tensor_mul(out=v, in0=qxyt, in1=cyxr[:, :kg])
                return v

            u0t = emit_u(0, GROUPS[0], groups[0][3])
            nc.vector.tensor_scalar_mul(
                out=cyxr[:, :, 0, :], in0=bcast(cyf), scalar1=-1.0)
            nc.vector.tensor_copy(out=cyxr[:, :, 1, :], in_=bcast(cxf))
            uvs.append((u0t, emit_v(0, GROUPS[0], groups[0][3])))
            nc.vector.tensor_scalar_mul(
                out=txr[:], in0=bcast(nn), scalar1=-1.0)
            nc.vector.tensor_scalar_mul(
                out=npzr[:], in0=bcast(pzs), scalar1=-1.0)
            for (g, kg, ks, qxyt, qzt) in groups[1:]:
                uvs.append((emit_u(g, kg, qxyt), emit_v(g, kg, qxyt)))

            # PE combines + ACT evictions + out-DMAs, group-major
            for (g, kg, ks, qxyt, qzt) in groups:
                u, v = uvs[g]
                oxyt = outpool.tile([P, KG, 2, T], BF16, tag="oxy",
                                    name=f"oxy{g}")[:, :kg]
                ozt = outpool.tile([P, KG, T], BF16, tag="oz",
                                   name=f"oz{g}")[:, :kg]
                width = kg * T
                npst = width // PST     # psum tiles per plane
                # per plane: list of (srcs_for_each_chunk, evict_dst_flat)
                u0 = u[:, :, 0, :]
                u1 = u[:, :, 1, :]
                v0 = v[:, :, 0, :]
                v1 = v[:, :, 1, :]
                planes = [
                    ("y", (v0, v1), oxyt[:, :, 1, :]),
                    ("z", (qzt, npzr), ozt),
                    ("x", (u0, u1, txr), oxyt[:, :, 0, :]),
                ]
                for pname, srcs, dst in planes:
                    for pt in range(npst):
                        ps = zpool.tile([P, PST], F32, tag="ps",
                                        name=f"ps_{pname}{g}_{pt}")
                        for ck in range(PST // BANK):
                            # 512-col chunk = 4 k-slabs of T
                            kc0 = pt * (PST // T) + ck * (BANK // T)
                            kcs = slice(kc0, kc0 + BANK // T)
                            pchunk = ps[:, ck * BANK:(ck + 1) * BANK]
                            nsrc = len(srcs)
                            for si, src in enumerate(srcs):
                                # REP tiles are k-replicated: any k window
                                sl = src[:, kcs, :] if src.shape[1] >= kc0 + \
                                    BANK // T else src[:, 0:BANK // T, :]
                                nc.tensor.matmul(
                                    pchunk, idpt[:], sl,
                                    start=(si == 0), stop=(si == nsrc - 1))
                            del pchunk
                        kw = slice(pt * (PST // T), (pt + 1) * (PST // T))
                        nc.scalar.copy(
                            out=dst[:, kw, :],
                            in_=ps[:].rearrange("p (k t) -> p k t", t=T))
                nc.scalar.dma_start(out=oz[:][:, ks, :], in_=ozt)
                nc.scalar.dma_start(out=oxy[:][:, ks, :, :], in_=oxyt)

    nc.compile()
    return nc


def _get_nc():
    if "nc" not in _CACHE:
        _CACHE["nc"] = _build_nc()
    return _CACHE["nc"]


_EYE = None


def make_in_maps(points, neighborhoods):
    """Host-side sharding + layout: per core, f32 point component planes
    [P, T], bf16 k-major xy-paired plane [P, K, 2, T], z plane [P, K, T],
    and the identity matmul stationary."""
    global _EYE
    pts = np.ascontiguousarray(np.asarray(points, dtype=np.float32))
    nb = np.asarray(neighborhoods, dtype=np.float32)
    assert pts.shape == (B, N, 3), pts.shape
    assert nb.shape == (B, N, K, 3), nb.shape

    if _EYE is None:
        _EYE = np.eye(P, dtype=BF16_NP)
    nb_bf = nb.astype(BF16_NP)
    # [B, NCORES, S, T, K, 3]
    nb_r = nb_bf.reshape(B, NCORES, S, T, K, 3)
    pts_r = pts.reshape(B, NCORES, S, T, 3)

    in_maps = []
    for c in range(NCORES):
        m = {"idp": _EYE}
        for ci, name in enumerate(("px", "py", "pz")):
            m[name] = np.ascontiguousarray(
                pts_r[:, c, :, :, ci].reshape(P, T))
        # [B, S, T, K, 2] -> [P, K, 2, T]
        m["qxy"] = np.ascontiguousarray(
            nb_r[:, c, :, :, :, 0:2].reshape(P, T, K, 2).transpose(0, 2, 3, 1))
        m["qz"] = np.ascontiguousarray(
            nb_r[:, c, :, :, :, 2].reshape(P, T, K).swapaxes(1, 2))
        in_maps.append(m)
    return in_maps


def assemble_out(results):
    """Merge per-core bf16 output planes back to [B, N, K, 3] f32."""
    out = np.empty((B, N, K, 3), dtype=np.float32)
    out_r = out.reshape(B, NCORES, S, T, K, 3)
    for c in range(NCORES):
        # [P, K, 2, T] -> [B, S, T, K, 2]
        oxyp = results[c]["oxy"].reshape(P, K, 2, T).transpose(0, 3, 1, 2)
        out_r[:, c, :, :, :, 0:2] = oxyp.reshape(B, S, T, K, 2)
        ozp = results[c]["oz"].reshape(P, K, T).swapaxes(1, 2)
        out_r[:, c, :, :, :, 2] = ozp.reshape(B, S, T, K)
    return out


def kernel(points, neighborhoods):
    in_maps = make_in_maps(points, neighborhoods)
    res = run_bass_kernel_spmd(_get_nc(), in_maps, list(range(NCORES))).results
    return assemble_out(res)
